# revision 38
# baseline (speedup 1.0000x reference)
"""Trainium2 Bass kernel for nn_EnhancedFractionalPINO.

Math rewrite (host-side, exact):
  * GL fractional conv is linear -> folded into Ws1:
      W1'[t,m] = sum_j w_j Ws1[t+j, m]  (causal correlation), plus a
      tail matrix for the cross-batch halo contribution.
  * spectral L2 and neural L1 have no nonlinearity between them:
      W23 = Ws2 @ Wn1 (512x512), b23 = bs2 @ Wn1 + bn1.
  * ifft2 is linear -> folded into Wn3:  G[f,:] = Re(ifft2(Wn3[f] img)).
  * Mirror symmetry: f = Re(fft2(x)) and out = Re(ifft2(proc)) satisfy
    a[u,v] == a[(64-u)%64, (64-v)%64], so only u-freq rows 0..32 are kept
    (34-row slab, row 33 = pad). W1' rows pair-fold on the host; G ships
    only stored-pixel columns; the host mirrors the output back to 64x64.
    This halves the two big matrices and the output traffic.

Kernel per core (batch-parallel, 32 batches/core):
  fft2 (rows 0..33 only) of 96+halo images via DFT matmuls -> fbuf
  h0 = fg @ W1g + tail @ WtailG  (both fp8-e3m4; per-col scales folded
                                  into the relu activation's scale AP)
  h1 = relu(h0 @ W23 + b23); h2 = relu(h1 @ W4 + b4)
  outg = h2 @ Gg + gbg           (Gg fp8-e3m4, per-row scales folded into
                                  L4's activation scale; ifft2 pre-applied)
  All matmuls feature-major: weights stationary (lhsT), batch=32 moving.
  Output leaves as [128, 51chunk, 32batch] f16; host unstages + mirrors.
"""

import numpy as np
import ml_dtypes

import concourse.bass as bass
import concourse.mybir as mybir
import concourse.tile as tile
from concourse import bacc
from concourse.bass_utils import run_bass_kernel_spmd

F32 = mybir.dt.float32
F16 = mybir.dt.float16
F8E3 = mybir.dt.float8e3
AF = mybir.ActivationFunctionType

B, C, H, W = 256, 3, 64, 64
MODES = C * H * W              # 12288
ALPHA = 0.5
NTOT = B * MODES
NCORE = 8
BS = B // NCORE                # 32 batches per core
NIMG = BS * C                  # 96 images per core
NSLOT = NIMG + 2               # halo + 96 images + zero pad
KTAPS = 512                    # truncated GL taps
KEEP = 34                      # stored u-freq rows per image (33 = pad)
ICOL = 17                      # fbuf cols per image (KEEP*64/128)
NCHG = 3 * ICOL                # 51 grouped chunks per batch
NG = NCHG * 128                # 6528 grouped positions per batch
NTL = 2                        # tail k-tiles (128-lag truncated tail)

GAM3 = 4.0                     # stage = out * GAM3 (host divides back)
BET = 1.0 / 16                 # h1 storage scale
T1, T2 = 13.9, 13.7            # e3m4 scale targets (tuned by scan)

MIRV = (64 - np.arange(64)) % 64


# ---------------------------------------------------------------- host consts
def _dft_consts():
    jk = np.outer(np.arange(64), np.arange(64)).astype(np.float64)
    Cm = np.cos(2 * np.pi * jk / 64)
    Sm = np.sin(2 * np.pi * jk / 64)
    # [cswi(68) | cmf(64) | msf(64) | ones(row0, 32)] packed into one DMA
    pack = np.zeros((64, 256))
    pack[:, 0:34] = Cm[:, 0:34]
    pack[:, 34:68] = Sm[:, 0:34]
    pack[:, 68:132] = Cm / 8
    pack[:, 132:196] = -Sm / 8
    pack[0, 196:196 + BS] = 1.0
    return {"fftpk": np.ascontiguousarray(pack, dtype=np.float16)}


def _gl_w():
    j = np.arange(1, KTAPS, dtype=np.float64)
    return np.concatenate([[1.0], np.cumprod((j - 1.0 - ALPHA) / j)])


def _col_major(Wm, ktiles):
    """[K, M] -> [128, ktiles, M] with partition = K % 128."""
    K, M = Wm.shape
    assert K == ktiles * 128
    return np.ascontiguousarray(Wm.reshape(ktiles, 128, M).transpose(1, 0, 2))


def _pm(v):
    """[512] -> [128, 4] f32 (partition, m-tile)."""
    return np.ascontiguousarray(v.reshape(4, 128).T, dtype=np.float32)


def _prep_weights(Ws1, bs1, Ws2, bs2, Wn1, bn1, Wn2, bn2, Wn3, bn3):
    w = _gl_w()
    hscale = (1.0 / (NTOT - 1)) ** (-ALPHA)
    W1 = Ws1.astype(np.float64) * hscale

    L = 1 << 15
    wf = np.fft.rfft(w, L).conj()[:, None]
    W1p = np.fft.irfft(np.fft.rfft(W1, L, axis=0) * wf, L, axis=0)[:MODES]
    Wtail = np.zeros((KTAPS, 512))
    for p in range(1, KTAPS):
        Wtail[p] = w[KTAPS - p:] @ W1[:p]

    W23 = Ws2.astype(np.float64) @ Wn1.astype(np.float64)
    b23 = bs2.astype(np.float64) @ Wn1.astype(np.float64) + bn1
    G = np.real(np.fft.ifft2(Wn3.astype(np.float64).reshape(512, 3, 64, 64),
                             axes=(-2, -1))).reshape(512, 3, 64, 64)
    gb = np.real(np.fft.ifft2(bn3.astype(np.float64).reshape(3, 64, 64),
                              axes=(-2, -1)))

    # mirror-fold W1' rows onto the stored set (k <= 32; k==33 is pad)
    W1g = np.zeros((NG, 512))
    W4d = W1p.reshape(3, 64, 64, 512)
    for c in range(3):
        for k in range(33):
            r0 = c * KEEP * 64 + k * 64
            W1g[r0:r0 + 64] = W4d[c, k]
            if 1 <= k <= 31:
                W1g[r0:r0 + 64] += W4d[c, 64 - k][MIRV]
    # tail matrix remapped to stored mirror coords: orig u=56+kk -> u'=8-kk
    WtG = np.zeros((NTL * 128, 512))
    for kk in (6, 7):   # keep lags <= 128 from the batch boundary
        WtG[(8 - kk) * 64 + MIRV] = Wtail[kk * 64:(kk + 1) * 64]

    # grouped G columns / bias (stored pixels only; pad cols zero)
    Gg = np.zeros((512, NG))
    gbg = np.zeros(NG)
    for c in range(3):
        Gg[:, c * 2176:c * 2176 + 33 * 64] = G[:, c, :33].reshape(512, -1)
        gbg[c * 2176:c * 2176 + 33 * 64] = gb[c, :33].reshape(-1)

    s1 = np.abs(W1g).max(axis=0) / T1
    w1d = _col_major((W1g / s1).astype(ml_dtypes.float8_e3m4),
                     NCHG).view(np.uint8)
    wtld = _col_major((WtG / s1[None, :]).astype(ml_dtypes.float8_e3m4),
                      NTL).view(np.uint8)
    sq = np.abs(Gg).max(axis=1) / T2
    gd = np.ascontiguousarray(
        (Gg / sq[:, None]).astype(ml_dtypes.float8_e3m4)
        .reshape(4, 128, NCHG, 128).transpose(1, 2, 0, 3)).view(np.uint8)

    # storage scales: fbuf = f/8 (folded into cmf/msf), a1 = relu(h0)/(8 s1),
    # h1 = relu(.)*BET, h2 = relu(.)*GAM3*sq -> all relus are plain copies
    f16 = lambda a: np.ascontiguousarray(a, dtype=np.float16)
    delta = GAM3 * sq
    W23s = W23 * (8.0 * s1)[:, None] * BET
    W4s = Wn2.astype(np.float64) / BET * delta[None, :]
    wpack = np.concatenate([_col_major(W23s, 4), _col_major(W4s, 4)],
                           axis=1)                          # [128, 8, 512]
    bias = np.concatenate([bs1 / (8.0 * s1), b23 * BET, bn2 * delta,
                           np.ones(BS)])
    return {
        "w1q": w1d,
        "wtl8": wtld,
        "wpk": f16(wpack),
        "gq": gd,
        "bias": np.ascontiguousarray(bias.reshape(1, -1), dtype=np.float32),
        "gb": f16((gbg * GAM3).reshape(1, NG)),
    }


# ---------------------------------------------------------------- bass module
_NC_CACHE = {}


def _build_nc():
    nc = bacc.Bacc("TRN2", target_bir_lowering=False, debug=False,
                   num_devices=NCORE)

    def din(name, shape, dt=F16):
        return nc.dram_tensor(name, shape, dt, kind="ExternalInput")

    d_x = din("ximgs", (64, NSLOT, 64))
    d_fpk = din("fftpk", (64, 256))
    d_w1 = din("w1q", (128, NCHG, 512), F8E3)
    d_wtl = din("wtl8", (128, NTL, 512), F8E3)
    d_wpk = din("wpk", (128, 8, 512))
    d_g = din("gq", (128, NCHG, 4, 128), F8E3)
    d_bias = din("bias", (1, 3 * 512 + BS), F32)
    d_gb = din("gb", (1, NG))
    d_out = nc.dram_tensor("out", (128, NCHG, BS), F16, kind="ExternalOutput")

    with tile.TileContext(nc) as tc:
        with tc.tile_pool(name="cpool", bufs=1) as cpool, \
             tc.tile_pool(name="bigpool", bufs=1) as bigpool:
            fpk = cpool.tile([64, 256], F16, tag="fpk")
            biasr = cpool.tile([1, 3 * 512 + BS], F32, tag="biasr")
            gbs = cpool.tile([1, NG], F16, tag="gbs")
            cswi = fpk[:, 0:68]
            cmf, msf = fpk[:, 68:132], fpk[:, 132:196]
            ones1 = fpk[0:1, 196:196 + BS]
            b1r, b23r = biasr[0:1, 0:512], biasr[0:1, 512:1024]
            b4r = biasr[0:1, 1024:1536]
            ones32 = biasr[0:1, 1536:1536 + BS]
            # big DMAs in priority order on the sync queue; small packs on
            # the scalar queue so they don't hold up the stream
            xall = bigpool.tile([64, NSLOT, 64], F16, tag="xall")
            nc.sync.dma_start(xall[:], d_x[:])
            nc.scalar.dma_start(fpk[:], d_fpk[:])
            nc.scalar.dma_start(biasr[:], d_bias[:])
            nc.scalar.dma_start(gbs[:], d_gb[:])

            w1s = bigpool.tile([128, NCHG, 512], F8E3, tag="w1s")
            for ch in range(3):
                nc.sync.dma_start(w1s[:, 17 * ch:17 * (ch + 1), :],
                                  d_w1[:, 17 * ch:17 * (ch + 1), :])
            wtl = bigpool.tile([128, NTL, 512], F8E3, tag="wtl")
            nc.sync.dma_start(wtl[:], d_wtl[:])
            wpk = bigpool.tile([128, 8, 512], F16, tag="wpk")
            nc.sync.dma_start(wpk[:], d_wpk[:])
            w23s = wpk[:, 0:4, :]
            w4s = wpk[:, 4:8, :]
            gs = bigpool.tile([128, NCHG, 4, 128], F8E3, tag="gs")
            gchunks = [(8 * i, 8 * (i + 1)) for i in range(5)] + \
                      [(40, 44), (44, 48), (48, 51)]
            for c0, c1 in gchunks:
                nc.sync.dma_start(gs[:, c0:c1, :, :], d_g[:, c0:c1, :, :])

            fbuf = bigpool.tile([128, NSLOT * ICOL], F16, tag="fbuf")
            a1 = bigpool.tile([128, 4, BS], F16, tag="a1")
            h1 = bigpool.tile([128, 4, BS], F16, tag="h1")
            h2 = bigpool.tile([128, 4, BS], F16, tag="h2")
            stage = bigpool.tile([128, NCHG, BS], F16, tag="stage")

            # ==== phase F: fft2 (u-freq rows 0..33 only) -> fbuf ===========
            with tc.tile_pool(name="gpool", bufs=6) as gpool, \
                 tc.tile_pool(name="ps1p", bufs=4, space="PSUM") as ps1p, \
                 tc.tile_pool(name="ps2p", bufs=3, space="PSUM") as ps2p:
                for grp in range(25):
                    n = 4 if grp < 24 else 2
                    psA = ps1p.tile([64, 272], F32, tag="psA")
                    for t in range(n):
                        i = grp * 4 + t
                        nc.tensor.matmul(psA[:, t * 68:(t + 1) * 68],
                                         xall[:, i, :], cswi,
                                         start=True, stop=True)
                    g1w = gpool.tile([64, 4, 68], F16, tag="g1w")
                    g1f = g1w[:, 0:n, :].rearrange("p a k -> p (a k)")
                    if grp % 2 == 0:
                        nc.scalar.copy(g1f, psA[:, 0:n * 68])
                    else:
                        nc.vector.tensor_copy(g1f, psA[:, 0:n * 68])
                    ps2 = ps2p.tile([64, 136], F32, tag="ps2")
                    nc.tensor.matmul(ps2[:, 0:n * 34], cmf,
                                     g1w[:, 0:n, 0:34], start=True, stop=False)
                    nc.tensor.matmul(ps2[:, 0:n * 34], msf,
                                     g1w[:, 0:n, 34:68], start=False,
                                     stop=True)
                    p2v = ps2.rearrange("p (i j two) -> p i j two",
                                        i=4, two=2)[:, 0:n]
                    base = grp * 4 * ICOL
                    cp = (nc.vector.tensor_copy if grp % 2 == 0
                          else nc.scalar.copy)
                    cp(fbuf[0:64, base:base + n * ICOL],
                       p2v[:, :, :, 0].rearrange("p i j -> p (i j)"))
                    cp(fbuf[64:128, base:base + n * ICOL],
                       p2v[:, :, :, 1].rearrange("p i j -> p (i j)"))

            fview = fbuf[:, ICOL:ICOL + BS * NCHG].rearrange(
                "p (b k) -> p b k", b=BS)
            ftail = fbuf[:, 0:BS * NCHG].rearrange("p (b k) -> p b k", b=BS)

            # ==== L1: h0 = fg @ W1g + tail, relu ===========================
            with tc.tile_pool(name="ps1m", bufs=1, space="PSUM") as ps1m:
                psL = [ps1m.tile([128, BS], F32, tag=f"psL{m}",
                                 name=f"psL{m}") for m in range(4)]
                for j in range(NCHG):
                    for m in range(4):
                        nc.tensor.matmul(psL[m][:],
                                         w1s[:, j, m * 128:(m + 1) * 128],
                                         fview[:, :, j],
                                         start=(j == 0), stop=False)
                for jt in range(NTL):
                    for m in range(4):
                        nc.tensor.matmul(psL[m][:],
                                         wtl[:, jt, m * 128:(m + 1) * 128],
                                         ftail[:, :, jt],
                                         start=False, stop=False)
                for m in range(4):
                    nc.tensor.matmul(psL[m][:],
                                     b1r[0:1, m * 128:(m + 1) * 128],
                                     ones32, start=False, stop=True)
                for m in range(4):
                    if m % 2 == 0:
                        nc.scalar.activation(a1[:, m, :], psL[m][:], AF.Relu)
                    else:
                        nc.vector.tensor_scalar_max(a1[:, m, :], psL[m][:],
                                                    0.0)

            # ==== L23 / L4: 512x512 layers =================================
            with tc.tile_pool(name="ps2m", bufs=4, space="PSUM") as ps2m:
                for src_t, dst, brow in ((a1, h1, b23r), (h1, h2, b4r)):
                    for m in range(4):
                        acc = ps2m.tile([128, BS], F32, tag="accm")
                        for k in range(4):
                            nc.tensor.matmul(
                                acc[:],
                                (w23s if dst is h1 else w4s)[
                                    :, k, m * 128:(m + 1) * 128],
                                src_t[:, k, :],
                                start=(k == 0), stop=False)
                        nc.tensor.matmul(acc[:],
                                         brow[0:1, m * 128:(m + 1) * 128],
                                         ones32, start=False, stop=True)
                        if m % 2 == 0:
                            nc.scalar.activation(dst[:, m, :], acc[:],
                                                 AF.Relu)
                        else:
                            nc.vector.tensor_scalar_max(dst[:, m, :],
                                                        acc[:], 0.0)

            # ==== L5: outg = h2 @ Gg + gbg (ifft2 pre-folded) ==============
            with tc.tile_pool(name="ps5m", bufs=4, space="PSUM") as ps5m:
                ogroups = {3: (0, 16), 10: (16, 43), 12: (43, 51)}
                for cg in range(13):
                    nsub = 4 if cg < 12 else 3
                    acc = ps5m.tile([128, 128], F32, tag="acc5")
                    for sub in range(nsub):
                        c = 4 * cg + sub
                        sl = acc[:, sub * 32:(sub + 1) * 32]
                        for k in range(4):
                            nc.tensor.matmul(sl, gs[:, c, k, :], h2[:, k, :],
                                             start=(k == 0), stop=False)
                        nc.tensor.matmul(sl,
                                         gbs[0:1, c * 128:(c + 1) * 128],
                                         ones1, start=False, stop=True)
                    dst = stage[:, 4 * cg:4 * cg + nsub, :].rearrange(
                        "p a b -> p (a b)")
                    if cg % 2 == 0:
                        nc.scalar.copy(dst, acc[:, 0:nsub * 32])
                    else:
                        nc.vector.tensor_copy(dst, acc[:, 0:nsub * 32])
                    if cg in ogroups:
                        c0, c1 = ogroups[cg]
                        nc.sync.dma_start(d_out[:, c0:c1, :],
                                          stage[:, c0:c1, :])

    nc.compile()
    return nc


def _get_nc():
    if "nc" not in _NC_CACHE:
        _NC_CACHE["nc"] = _build_nc()
    return _NC_CACHE["nc"]


def _make_in_maps(x, Ws1, bs1, Ws2, bs2, Wn1, bn1, Wn2, bn2, Wn3, bn3):
    shared = dict(_dft_consts())
    shared.update(_prep_weights(Ws1, bs1, Ws2, bs2, Wn1, bn1, Wn2, bn2,
                                Wn3, bn3))
    in_maps = []
    for g in range(NCORE):
        if g == 0:
            halo = np.zeros((1, 64, 64), np.float32)
        else:
            halo = x[g * BS - 1, 2][None]
        ximgs = np.concatenate(
            [halo, x[g * BS:(g + 1) * BS].reshape(NIMG, 64, 64),
             np.zeros((1, 64, 64), np.float32)]).astype(np.float16)
        in_maps.append({"ximgs": np.ascontiguousarray(
            ximgs.transpose(1, 0, 2)), **shared})
    return in_maps


def _unstage(st):
    """[128, NCHG, BS] staged f16 -> [BS, C, H, W] f32 (mirror-expanded)."""
    og = (np.asarray(st, dtype=np.float32) / GAM3).transpose(2, 1, 0)
    og = og.reshape(BS, C, KEEP, 64)
    out = np.empty((BS, C, H, W), np.float32)
    out[:, :, :33] = og[:, :, :33]
    out[:, :, 33:] = og[:, :, np.arange(31, 0, -1)][:, :, :, MIRV]
    return out


def kernel(**inputs):
    ins = {k: np.asarray(v) for k, v in inputs.items()}
    x = np.ascontiguousarray(ins["x"], dtype=np.float32)
    nc = _get_nc()
    in_maps = _make_in_maps(
        x, ins["Ws1"], ins["bs1"], ins["Ws2"], ins["bs2"],
        ins["Wn1"], ins["bn1"], ins["Wn2"], ins["bn2"],
        ins["Wn3"], ins["bn3"])
    res = run_bass_kernel_spmd(nc, in_maps, list(range(NCORE)))
    out = np.empty((B, C, H, W), np.float32)
    for g in range(NCORE):
        out[g * BS:(g + 1) * BS] = _unstage(res.results[g]["out"])
    return out
